# revision 54
# baseline (speedup 1.0000x reference)
"""BesselKAN layer kernel for Trainium2 (8 NeuronCores, data-parallel over batch).

reference math:
    t = tanh(x)                                   # [B, I]
    b0 = 1; b1 = t + 1; b2 = 3t*b1 + b0; b3 = 5t*b2 + b1
    y[b,o] = sum_{i,d} b_d[b,i] * W[i,o,d]        # W = bessel_coeffs [I, O, 4]

Monomial rewrite (exact algebra):
    b0 = 1
    b1 = 1 + t
    b2 = 1 + 3t + 3t^2
    b3 = 1 + 6t + 15t^2 + 15t^3
    y  = 1 @ (W0+W1+W2+W3)            -> bias row, rank-1: bias_o = colsum
       + t        @ (W1 + 3W2 + 6W3)  =: u1 @ C1
       + (3 t^2)  @ (W2 + 5W3)        =: u2 @ C2
       + (15 t^3) @ W3                =: u3 @ C3

So per core (1024 batch rows): 3 bf16 matmuls of [1024,1024]x[1024,1024]
plus a broadcast bias add.  The bias is computed with an all-ones f32r
stationary matmul over the raw fp32 W chunks (ones^T @ W broadcasts the
column-sum into every PSUM partition).
"""

import os
import sys

import numpy as np

if "/opt/trn_rl_repo" not in sys.path:
    sys.path.insert(0, "/opt/trn_rl_repo")

from contextlib import ExitStack

import concourse.bass as bass
import concourse.tile as tile
from concourse import bacc, mybir
from concourse._compat import with_exitstack
from concourse.masks import make_identity

P = 128
N_CORES = 8
B_FULL = 8192
I_DIM = 1024
O_DIM = 1024
NDEG = 4  # D+1

FP32 = mybir.dt.float32
F32R = mybir.dt.float32r
BF16 = mybir.dt.float16


DEFAULT_CFG = dict(
    nsplit=2,       # x DMA/tanh split per row-block
    ga=2,           # leading bi-group riding the W stream
    d2=0, d3=0,     # term pipeline delays for u2/u3 (u's are ready early)
    w_pre=True,     # issue first W chunk right after x0
    n_x_early=4,    # row-blocks emitted before the W loop
    u3_k1_double=True,  # emit two u3 ops at ki=1
    j_ramp=2,       # u1 rows issued at chunk 0 (rest catch up at chunk 1)
    xbufs=3, wbufs=6, obufs=3, pobufs=4, tbufs=2, ptbufs=3,
)


@with_exitstack
def _bessel_body(ctx: ExitStack, tc: "tile.TileContext", y_d, x_d, w_d,
                 b_loc, i_dim, o_dim, cfg=None):
    cfg = {**DEFAULT_CFG, **(cfg or {})}
    nc = tc.nc
    BI = b_loc // P           # batch tiles
    KI = i_dim // P           # contraction tiles
    OW = min(512, o_dim)      # matmul moving free size (one PSUM bank fp32)
    OH = o_dim // OW          # output column tiles

    singles = ctx.enter_context(tc.tile_pool(name="singles", bufs=1))
    xpool = ctx.enter_context(tc.tile_pool(name="xpool", bufs=cfg["xbufs"]))
    tpool = ctx.enter_context(tc.tile_pool(name="tpool", bufs=cfg["tbufs"]))
    wpool = ctx.enter_context(tc.tile_pool(name="wpool", bufs=cfg["wbufs"]))
    opool = ctx.enter_context(tc.tile_pool(name="opool", bufs=cfg["obufs"]))
    psum_t = ctx.enter_context(
        tc.tile_pool(name="psum_t", bufs=cfg["ptbufs"], space="PSUM"))
    psum_b = ctx.enter_context(tc.tile_pool(name="psum_b", bufs=1, space="PSUM"))
    psum_o = ctx.enter_context(
        tc.tile_pool(name="psum_o", bufs=cfg["pobufs"], space="PSUM"))

    identity16 = singles.tile([P, P], BF16, name="identity16")
    make_identity(nc, identity16)
    # All-λ stationary matrices: (λ·ones)^T @ M broadcasts λ·colsum(M) into
    # every PSUM partition.  bias = colsum(W0) + colsum(C1 - 2*C2 + 5*C3)
    # because C1 - 2*C2 + 5*C3 == W1 + W2 + W3.
    ones_bf = singles.tile([P, P], BF16, name="ones_bf")
    neg2_bf = singles.tile([P, P], BF16, name="neg2_bf")
    five_bf = singles.tile([P, P], BF16, name="five_bf")
    nc.vector.memset(ones_bf[:], 1.0)
    nc.vector.memset(neg2_bf[:], -2.0)
    nc.vector.memset(five_bf[:], 5.0)

    # Persistent basis (u, [i_part, ki, b]) and combined weights (C, [i_part, ki, o]).
    u1 = singles.tile([P, KI, b_loc], BF16, name="u1")
    u2 = singles.tile([P, KI, b_loc], BF16, name="u2")
    u3 = singles.tile([P, KI, b_loc], BF16, name="u3")

    # ---- phase X: tanh (bf16) -> PE transpose -> u1; u2 = 3t^2 (GpSimd).
    # u3 = 5t*u2 is emitted later inside the W stream (DVE) so it neither
    # blocks C-prep ordering nor gates the early A-group matmuls.
    def emit_x_phase(bi):
        bsl = slice(bi * P, (bi + 1) * P)
        x_t = xpool.tile([P, i_dim], FP32, tag="x_t", name=f"x_t{bi}")
        tf = tpool.tile([P, i_dim], BF16, tag="tf", name=f"tf{bi}")
        nsplit = cfg["nsplit"] if bi == 0 else 1
        kstep = KI // nsplit
        for s in range(nsplit):
            ssl = slice(s * kstep * P, (s + 1) * kstep * P)
            nc.sync.dma_start(out=x_t[:, ssl],
                              in_=x_d[bi * P:(bi + 1) * P, ssl])
            nc.scalar.activation(out=tf[:, ssl], in_=x_t[:, ssl],
                                 func=mybir.ActivationFunctionType.Tanh)
            for ki in range(s * kstep, (s + 1) * kstep):
                ps = psum_t.tile([P, P], BF16, tag="ps_t",
                                 name=f"ps_t{bi}_{ki}")
                nc.tensor.transpose(ps[:], tf[:, ki * P:(ki + 1) * P],
                                    identity16[:])
                nc.scalar.copy(u1[:, ki, bsl], ps[:])
        u1s = u1[:, :, bsl]
        nc.vector.scalar_tensor_tensor(
            out=u2[:, :, bsl], in0=u1s, scalar=3.0, in1=u1s,
            op0=mybir.AluOpType.mult, op1=mybir.AluOpType.mult,
        )
        nc.vector.scalar_tensor_tensor(
            out=u3[:, :, bsl], in0=u1s, scalar=5.0, in1=u2[:, :, bsl],
            op0=mybir.AluOpType.mult, op1=mybir.AluOpType.mult,
        )

    def emit_u3(bi):
        bsl = slice(bi * P, (bi + 1) * P)
        nc.vector.scalar_tensor_tensor(
            out=u3[:, :, bsl], in0=u1[:, :, bsl], scalar=5.0,
            in1=u2[:, :, bsl],
            op0=mybir.AluOpType.mult, op1=mybir.AluOpType.mult,
        )

    def issue_w(oh, ki):
        w_t = wpool.tile([P, OW, NDEG], FP32, tag="w_t", name=f"w_t{oh}_{ki}")
        nc.sync.dma_start(
            out=w_t[:],
            in_=w_d[ki * P:(ki + 1) * P, oh * OW:(oh + 1) * OW, :])
        return w_t

    n_x_emitted = min(cfg["n_x_early"], BI)
    w_pre = None
    for bi in range(n_x_emitted):
        emit_x_phase(bi)
        if bi == 0 and cfg["w_pre"]:
            # first W chunk rides right behind x0 so C[0] is ready by the
            # time the transposes drain
            w_pre = issue_w(0, 0)
    u3_pending = []

    # ---- phases W+MAIN, one o-column half at a time so the second half's
    # W stream overlaps the first half's matmuls.  Separate C/bias tensors
    # per half avoid false WAR deps in Tile's access tracking.
    GA = min(cfg["ga"], BI)  # leading bi-group interleaved with W stream
    for oh in range(OH):
        osl = slice(oh * OW, (oh + 1) * OW)
        c1 = singles.tile([P, KI, OW], BF16, name=f"c1_{oh}")
        c2 = singles.tile([P, KI, OW], BF16, name=f"c2_{oh}")
        c3 = singles.tile([P, KI, OW], BF16, name=f"c3_{oh}")
        bias_ps = psum_b.tile([P, OW], FP32, tag="bias_ps",
                              name=f"bias_ps{oh}")
        bias = singles.tile([P, OW], FP32, name=f"bias{oh}")
        terms = ((u1, c1), (u2, c2), (u3, c3))
        pos_a = [psum_o.tile([P, OW], FP32, tag="po", name=f"po_a{oh}_{j}")
                 for j in range(GA)]
        D = (0, cfg["d2"], cfg["d3"])  # per-term chunk delays
        a_started = [False] * GA

        def a_mms(ti, kk, js):
            u, cc = terms[ti]
            for j in js:
                nc.tensor.matmul(
                    pos_a[j][:],
                    u[:, kk, j * P:(j + 1) * P],
                    cc[:, kk, :],
                    start=not a_started[j],
                    stop=(ti == len(terms) - 1 and kk == KI - 1),
                )
                a_started[j] = True

        def a_step(c):
            # software-pipelined A-group accumulation step for chunk index c.
            # At c=0 (first half only) just j0/j1 — later rows' tanh results
            # aren't in yet and an in-order PE stall would block everything;
            # the skipped pairs catch up at c=1.
            for ti, (u, cc) in enumerate(terms):
                kk = c - D[ti]
                if not 0 <= kk < KI:
                    continue
                jr = cfg["j_ramp"]
                if ti == 0 and c == 0 and oh == 0 and GA > jr:
                    a_mms(ti, kk, range(jr))
                else:
                    a_mms(ti, kk, range(GA))
            if c == 1 and oh == 0 and GA > cfg["j_ramp"]:
                a_mms(0, 0, range(cfg["j_ramp"], GA))

        for ki in range(KI):
            if oh == 0 and ki >= 1 and u3_pending:
                emit_u3(u3_pending.pop(0))
                if ki == 1 and cfg["u3_k1_double"] and u3_pending:
                    emit_u3(u3_pending.pop(0))
            w_t = w_pre if (oh == 0 and ki == 0 and w_pre is not None) \
                else issue_w(oh, ki)
            w1 = w_t[:, :, 1]
            w2 = w_t[:, :, 2]
            w3 = w_t[:, :, 3]
            tmp = wpool.tile([P, OW], FP32, tag="tmpc")
            # c1 = w1 + 3*w2 + 6*w3 ; c2 = w2 + 5*w3 ; c3 = w3
            nc.vector.scalar_tensor_tensor(
                out=tmp[:], in0=w2, scalar=3.0, in1=w1,
                op0=mybir.AluOpType.mult, op1=mybir.AluOpType.add,
            )
            nc.vector.scalar_tensor_tensor(
                out=c1[:, ki, :], in0=w3, scalar=6.0, in1=tmp[:],
                op0=mybir.AluOpType.mult, op1=mybir.AluOpType.add,
            )
            nc.vector.scalar_tensor_tensor(
                out=c2[:, ki, :], in0=w3, scalar=5.0, in1=w2,
                op0=mybir.AluOpType.mult, op1=mybir.AluOpType.add,
            )
            nc.scalar.copy(c3[:, ki, :], w3)
            w0_bf = wpool.tile([P, OW], BF16, tag="w0_bf")
            nc.scalar.copy(w0_bf[:], w_t[:, :, 0])
            # leading bi-group rides the W stream (terms pipelined so late
            # u2/u3 availability never stalls the PE).
            a_step(ki)
            # bias accumulation over ki: colsum(W0) + colsum(C1 - 2*C2 + 5*C3)
            movers = (w0_bf[:], c1[:, ki, :], c2[:, ki, :], c3[:, ki, :])
            lhs = (ones_bf, ones_bf, neg2_bf, five_bf)
            for t in range(4):
                nc.tensor.matmul(
                    bias_ps[:],
                    lhs[t][:],
                    movers[t],
                    start=(ki == 0 and t == 0),
                    stop=(ki == KI - 1 and t == 3),
                )
        while oh == 0 and n_x_emitted < BI:
            emit_x_phase(n_x_emitted)
            n_x_emitted += 1
        while oh == 0 and u3_pending:
            emit_u3(u3_pending.pop(0))
        for c in range(KI, KI + D[-1]):
            a_step(c)
        nc.vector.tensor_copy(bias[:], bias_ps[:])
        for j in range(GA):
            yo = opool.tile([P, OW], FP32, tag="yo")
            nc.vector.tensor_add(yo[:], pos_a[j][:], bias[:])
            nc.sync.dma_start(out=y_d[j * P:(j + 1) * P, osl], in_=yo[:])

        # trailing bi-groups: all C for this half is resident, full speed.
        for bi in range(GA, BI):
            bsl = slice(bi * P, (bi + 1) * P)
            po = psum_o.tile([P, OW], FP32, tag="po")
            for ki in range(KI):
                for ti, (u, c) in enumerate(terms):
                    nc.tensor.matmul(
                        po[:],
                        u[:, ki, bsl],
                        c[:, ki, :],
                        start=(ki == 0 and ti == 0),
                        stop=(ki == KI - 1 and ti == len(terms) - 1),
                    )
            yo = opool.tile([P, OW], FP32, tag="yo")
            nc.vector.tensor_add(yo[:], po[:], bias[:])
            nc.sync.dma_start(out=y_d[bi * P:(bi + 1) * P, osl], in_=yo[:])


def build_nc(b_loc=B_FULL // N_CORES, i_dim=I_DIM, o_dim=O_DIM,
             n_cores=N_CORES, cfg=None):
    nc = bacc.Bacc("TRN2", target_bir_lowering=False, debug=False,
                   num_devices=n_cores)
    x_d = nc.dram_tensor("x", [b_loc, i_dim], FP32, kind="ExternalInput").ap()
    w_d = nc.dram_tensor("w", [i_dim, o_dim, NDEG], FP32,
                         kind="ExternalInput").ap()
    y_d = nc.dram_tensor("y", [b_loc, o_dim], FP32, kind="ExternalOutput").ap()
    with tile.TileContext(nc) as tc:
        _bessel_body(tc, y_d, x_d, w_d, b_loc, i_dim, o_dim, cfg=cfg)
    nc.compile()
    return nc


_NC_CACHE = {}


def _get_nc():
    key = "full"
    if key not in _NC_CACHE:
        _NC_CACHE[key] = build_nc()
    return _NC_CACHE[key]


def run_spmd(x, bessel_coeffs, trace=False, **kwargs):
    """Run the SPMD kernel on 8 cores; returns (y_full, BassKernelResults)."""
    from concourse.bass_utils import run_bass_kernel_spmd

    nc = _get_nc()
    x = np.ascontiguousarray(np.asarray(x, dtype=np.float32))
    w = np.ascontiguousarray(np.asarray(bessel_coeffs, dtype=np.float32))
    b_loc = x.shape[0] // N_CORES
    in_maps = [
        {"x": x[c * b_loc:(c + 1) * b_loc], "w": w} for c in range(N_CORES)
    ]
    res = run_bass_kernel_spmd(nc, in_maps, core_ids=list(range(N_CORES)),
                               trace=trace, **kwargs)
    y = np.concatenate([r["y"] for r in res.results], axis=0)
    return y, res


def kernel(x, bessel_coeffs):
    y, _ = run_spmd(x, bessel_coeffs)
    return y.astype(np.float32)


def _ref_np(x, w):
    t = np.tanh(np.asarray(x, dtype=np.float64))
    w = np.asarray(w, dtype=np.float64)
    basis = [np.ones_like(t), t + 1.0]
    for i in range(2, NDEG):
        basis.append((2 * i - 1) * t * basis[i - 1] + basis[i - 2])
    bz = np.stack(basis, axis=-1)
    return np.einsum("bid,iod->bo", bz, w)


def _selftest_sim(b_loc=256, i_dim=256, o_dim=1024):
    """CoreSim check on a small config exercising all loop paths."""
    from concourse.bass_interp import CoreSim

    nc = build_nc(b_loc=b_loc, i_dim=i_dim, o_dim=o_dim, n_cores=1)
    rng = np.random.default_rng(0)
    x = rng.standard_normal((b_loc, i_dim)).astype(np.float32)
    w = (rng.standard_normal((i_dim, o_dim, NDEG)) / (i_dim * NDEG)).astype(
        np.float32)
    sim = CoreSim(nc)
    sim.tensor("x")[:] = x
    sim.tensor("w")[:] = w
    sim.simulate()
    y = np.array(sim.tensor("y"))
    ref = _ref_np(x, w)
    scale = np.abs(ref).max()
    err = np.abs(y - ref).max() / scale
    print(f"sim scale={scale:.4g} max_abs_rel_err={err:.4g}")
    assert err < 2e-2, err
    print("SIM OK")


if __name__ == "__main__":
    if "--sim" in sys.argv:
        _selftest_sim()
